# revision 11
# baseline (speedup 1.0000x reference)
"""Trainium2 Bass kernel for nn_MAK_27401891348771 (gnn_message_passing).

Math (reference):
  t0 = lrelu(BN(W0 @ y));  t1 = lrelu(BN(Wm @ t0));  w = W1 @ t1
  out[b,n,k,o] = sum_{i,h} w[(o,i,h)][b,n,k] * x[b,i,n,k]
  out = lrelu(BN(out) + x)

Key algebraic folds used here:
  - H axis folded into weights on host: V[o,i,f] = sum_h W1[(o,i,h), f]
  - filter apply per point p: out[o,p] = sum_i x[i,p] * A[(o,i),p],
    A = V3 @ t1n  (PE matmul), the x multiply on DVE, the i-reduction as a
    PE matmul against a 0/1 selection mask with PSUM accumulation.
Sharding: N axis across 8 cores (5120 points/core); BN stats via tiny
AllReduce collectives (3x, 256B payloads).
"""

import os
import numpy as np

os.environ.setdefault("MYCRO_LOCAL_CACHE", "1")

B, Cin, Cout, Cfeat, N, K, H = 2, 32, 32, 64, 1024, 20, 4
NCORES = 8
NS = N // NCORES            # 128 n-values per core
P = B * NS * K              # 5120 points per core
PTOT = B * N * K            # 40960 points total
HP = P // 2                 # 2560, y half size
EPS = 1e-5
SLOPE = 0.2

_CACHE = {}
DEBUG_STAGES = False


def _build_program():
    import concourse.bass as bass
    import concourse.tile as tile
    import concourse.bacc as bacc
    from concourse import mybir

    f32 = mybir.dt.float32
    AF = mybir.ActivationFunctionType
    ALU = mybir.AluOpType

    nc = bacc.Bacc(
        "TRN2",
        target_bir_lowering=False,
        debug=False,
        enable_asserts=True,
        num_devices=NCORES,
    )

    # ---- DRAM I/O -------------------------------------------------------
    y0_d = nc.dram_tensor("y0", [64, HP], f32, kind="ExternalInput")
    y1_d = nc.dram_tensor("y1", [64, HP], f32, kind="ExternalInput")
    xr_d = nc.dram_tensor("xrep", [128, P], f32, kind="ExternalInput")
    w0t_d = nc.dram_tensor("w0t", [64, 32], f32, kind="ExternalInput")
    wmt_d = nc.dram_tensor("wmt", [32, 32], f32, kind="ExternalInput")
    v3t_d = nc.dram_tensor("v3t", [32, 1024], f32, kind="ExternalInput")
    sm_d = nc.dram_tensor("smask", [128, 256], f32, kind="ExternalInput")
    bnp_d = nc.dram_tensor("bnp", [32, 6], f32, kind="ExternalInput")
    out_d = nc.dram_tensor("out", [32, P], f32, kind="ExternalOutput")
    if DEBUG_STAGES:
        dbg_t0 = nc.dram_tensor("dbg_t0", [32, P], f32, kind="ExternalOutput")
        dbg_t0n = nc.dram_tensor("dbg_t0n", [32, P], f32, kind="ExternalOutput")
        dbg_t1n = nc.dram_tensor("dbg_t1n", [32, P], f32, kind="ExternalOutput")
        dbg_opre = nc.dram_tensor("dbg_opre", [32, P], f32, kind="ExternalOutput")
        dbg_st = nc.dram_tensor("dbg_st", [32, 12], f32, kind="ExternalOutput")

    RG = [list(range(NCORES))]

    with tile.TileContext(nc, num_cores=NCORES) as tc:
        with (
            tc.tile_pool(name="big", bufs=1) as big,
            tc.tile_pool(name="wts", bufs=1) as wts,
            tc.tile_pool(name="zp", bufs=6) as zp,
            tc.tile_pool(name="fin", bufs=4) as finp,
            tc.tile_pool(name="st", bufs=1) as stp,
            tc.tile_pool(name="psT", bufs=2, space="PSUM") as psT,
            tc.tile_pool(name="psA", bufs=3, space="PSUM") as psA,
            tc.tile_pool(name="psO", bufs=2, space="PSUM") as psO,
            tc.tile_pool(name="dram", bufs=1, space="DRAM") as dram,
        ):
            # ---- persistent SBUF tensors -------------------------------
            y0 = big.tile([64, HP], f32, tag="y0")
            y1 = big.tile([64, HP], f32, tag="y1")
            xr = big.tile([128, P], f32, tag="xr")
            t0 = big.tile([32, P], f32, tag="t0")
            t0n = big.tile([32, P], f32, tag="t0n")
            t1 = big.tile([32, P], f32, tag="t1")
            t1n = big.tile([32, P], f32, tag="t1n")
            opre = big.tile([32, P], f32, tag="opre")
            w0t = wts.tile([64, 32], f32, tag="w0t")
            wmt = wts.tile([32, 32], f32, tag="wmt")
            v3t = wts.tile([32, 1024], f32, tag="v3t")
            smk = wts.tile([128, 256], f32, tag="smk")
            bnp = wts.tile([32, 6], f32, tag="bnp")

            # ---- loads (split for DMA-queue parallelism) ---------------
            for c in range(4):
                nc.sync.dma_start(y0[:, c * 640:(c + 1) * 640],
                                  y0_d[:, c * 640:(c + 1) * 640])
                nc.sync.dma_start(y1[:, c * 640:(c + 1) * 640],
                                  y1_d[:, c * 640:(c + 1) * 640])
            for c in range(8):
                nc.sync.dma_start(xr[:, c * 640:(c + 1) * 640],
                                  xr_d[:, c * 640:(c + 1) * 640])
            nc.sync.dma_start(w0t[:], w0t_d[:])
            nc.sync.dma_start(wmt[:], wmt_d[:])
            nc.sync.dma_start(v3t[:], v3t_d[:])
            nc.sync.dma_start(smk[:], sm_d[:])
            nc.sync.dma_start(bnp[:], bnp_d[:])

            # ---- helpers -----------------------------------------------
            def stats(src, nchunks=10):
                """per-channel (sum, sumsq) over the free dim of src (32,P)."""
                parts = stp.tile([32, 16], f32, tag="sqparts")
                F = P // nchunks
                for c in range(nchunks):
                    scr = finp.tile([32, F], f32, tag="fin")
                    nc.scalar.activation(
                        out=scr[:], in_=src[:, c * F:(c + 1) * F],
                        func=AF.Square, accum_out=parts[:, c:c + 1])
                ssum = stp.tile([32, 1], f32, tag=f"ssum_{src.name}")
                ssq = stp.tile([32, 1], f32, tag=f"ssq_{src.name}")
                nc.vector.tensor_reduce(
                    out=ssum[:], in_=src[:], axis=mybir.AxisListType.X,
                    op=ALU.add)
                nc.vector.tensor_reduce(
                    out=ssq[:], in_=parts[:, 0:nchunks],
                    axis=mybir.AxisListType.X, op=ALU.add)
                return ssum, ssq

            def allreduce_stats(ssum, ssq, idx):
                """AllReduce (32,2) stats; returns SBUF (32,2) of global sums."""
                pack = stp.tile([32, 2], f32, tag=f"arpack{idx}")
                nc.vector.tensor_copy(pack[:, 0:1], ssum[:])
                nc.vector.tensor_copy(pack[:, 1:2], ssq[:])
                bin_ = dram.tile([32, 2], f32, tag=f"arin{idx}")
                bout = dram.tile([32, 2], f32, tag=f"arout{idx}")
                nc.gpsimd.dma_start(bin_[:], pack[:])
                nc.gpsimd.collective_compute(
                    "AllReduce", ALU.add, replica_groups=RG,
                    ins=[bin_.opt()], outs=[bout.opt()])
                glob = stp.tile([32, 2], f32, tag=f"arglob{idx}")
                nc.gpsimd.dma_start(glob[:], bout[:])
                return glob

            def bn_coeffs(glob, gcol, bcol, idx):
                """scale/bias from global (sum,sumsq): s=g*rsqrt(var+eps),
                b = beta - mean*s."""
                mean = stp.tile([32, 1], f32, tag=f"mean{idx}")
                e2 = stp.tile([32, 1], f32, tag=f"e2{idx}")
                nc.scalar.activation(out=mean[:], in_=glob[:, 0:1],
                                     func=AF.Copy, scale=1.0 / PTOT)
                nc.scalar.activation(out=e2[:], in_=glob[:, 1:2],
                                     func=AF.Copy, scale=1.0 / PTOT)
                m2 = stp.tile([32, 1], f32, tag=f"m2{idx}")
                nc.scalar.activation(out=m2[:], in_=mean[:], func=AF.Square)
                varp = stp.tile([32, 1], f32, tag=f"varp{idx}")
                # varp = (e2 - m2) + eps
                nc.vector.scalar_tensor_tensor(
                    out=varp[:], in0=e2[:], scalar=EPS, in1=m2[:],
                    op0=ALU.add, op1=ALU.subtract)
                rv = stp.tile([32, 1], f32, tag=f"rv{idx}")
                nc.vector.reciprocal(rv[:], varp[:])
                isd = stp.tile([32, 1], f32, tag=f"isd{idx}")
                nc.scalar.activation(out=isd[:], in_=rv[:], func=AF.Sqrt)
                s = stp.tile([32, 1], f32, tag=f"s{idx}")
                nc.vector.scalar_tensor_tensor(
                    out=s[:], in0=isd[:], scalar=1.0, in1=bnp[:, gcol:gcol + 1],
                    op0=ALU.mult, op1=ALU.mult)
                ms = stp.tile([32, 1], f32, tag=f"ms{idx}")
                nc.vector.scalar_tensor_tensor(
                    out=ms[:], in0=mean[:], scalar=-1.0, in1=s[:],
                    op0=ALU.mult, op1=ALU.mult)
                bia = stp.tile([32, 1], f32, tag=f"bia{idx}")
                nc.vector.scalar_tensor_tensor(
                    out=bia[:], in0=ms[:], scalar=0.0, in1=bnp[:, bcol:bcol + 1],
                    op0=ALU.add, op1=ALU.add)
                return s, bia

            # ---- phase 1: t0 = W0 @ y ----------------------------------
            for h, ysb in ((0, y0), (1, y1)):
                for c in range(5):
                    ps = psT.tile([32, 512], f32, tag="psT")
                    nc.tensor.matmul(ps[:], w0t[:], ysb[:, c * 512:(c + 1) * 512],
                                     start=True, stop=True)
                    nc.scalar.activation(
                        out=t0[:, h * HP + c * 512: h * HP + (c + 1) * 512],
                        in_=ps[:], func=AF.Copy)

            s0_sum, s0_sq = stats(t0)
            g0 = allreduce_stats(s0_sum, s0_sq, 0)
            s0, b0 = bn_coeffs(g0, 0, 1, 0)

            # ---- phase 2: t0n = lrelu(bn0(t0)); t1 = Wm @ t0n ----------
            for c in range(10):
                sl = slice(c * 512, (c + 1) * 512)
                aff = finp.tile([32, 512], f32, tag="fin")
                nc.scalar.activation(out=aff[:], in_=t0[:, sl],
                                     func=AF.Identity, scale=s0[:], bias=b0[:])
                nc.vector.scalar_tensor_tensor(
                    out=t0n[:, sl], in0=aff[:], scalar=SLOPE, in1=aff[:],
                    op0=ALU.mult, op1=ALU.max)
            for c in range(10):
                sl = slice(c * 512, (c + 1) * 512)
                ps = psT.tile([32, 512], f32, tag="psT")
                nc.tensor.matmul(ps[:], wmt[:], t0n[:, sl], start=True, stop=True)
                nc.scalar.activation(out=t1[:, sl], in_=ps[:], func=AF.Copy)

            s1_sum, s1_sq = stats(t1)
            g1 = allreduce_stats(s1_sum, s1_sq, 1)
            s1, b1 = bn_coeffs(g1, 2, 3, 1)

            # ---- phase 3: t1n; filter generate + apply ------------------
            for c in range(10):
                sl = slice(c * 512, (c + 1) * 512)
                aff = finp.tile([32, 512], f32, tag="fin")
                nc.scalar.activation(out=aff[:], in_=t1[:, sl],
                                     func=AF.Identity, scale=s1[:], bias=b1[:])
                nc.vector.scalar_tensor_tensor(
                    out=t1n[:, sl], in0=aff[:], scalar=SLOPE, in1=aff[:],
                    op0=ALU.mult, op1=ALU.max)

            # per group g of 1280 points, col tiles of 512/512/256
            for g in range(4):
                base = g * 1280
                for c0, F in ((0, 512), (512, 512), (1024, 256)):
                    sl = slice(base + c0, base + c0 + F)
                    zs = []
                    for m in range(8):
                        a_ps = psA.tile([128, 512], f32, tag="psA")
                        nc.tensor.matmul(
                            a_ps[:, 0:F], v3t[:, m * 128:(m + 1) * 128],
                            t1n[:, sl], start=True, stop=True)
                        z = zp.tile([128, 512], f32, tag="z")
                        # z = A * xrep
                        nc.vector.scalar_tensor_tensor(
                            out=z[:, 0:F], in0=a_ps[:, 0:F], scalar=1.0,
                            in1=xr[:, sl], op0=ALU.mult, op1=ALU.mult)
                        zs.append(z)
                    o_ps = psO.tile([32, 512], f32, tag="psO")
                    for m in range(8):
                        nc.tensor.matmul(
                            o_ps[:, 0:F], smk[:, m * 32:(m + 1) * 32],
                            zs[m][:, 0:F], start=(m == 0), stop=(m == 7))
                    nc.scalar.activation(out=opre[:, sl], in_=o_ps[:, 0:F],
                                         func=AF.Copy)

            s2_sum, s2_sq = stats(opre)
            g2 = allreduce_stats(s2_sum, s2_sq, 2)
            s2, b2 = bn_coeffs(g2, 4, 5, 2)

            if DEBUG_STAGES:
                for c in range(4):
                    sl = slice(c * 1280, (c + 1) * 1280)
                    nc.sync.dma_start(dbg_t0[:, sl], t0[:, sl])
                    nc.sync.dma_start(dbg_t0n[:, sl], t0n[:, sl])
                    nc.sync.dma_start(dbg_t1n[:, sl], t1n[:, sl])
                    nc.sync.dma_start(dbg_opre[:, sl], opre[:, sl])
                stt = stp.tile([32, 12], f32, tag="dbgst")
                for j, ap in enumerate((g0, s0, b0, g1, s1, b1, g2, s2, b2)):
                    w = ap.shape[1] if len(ap.shape) > 1 else 1
                    nc.vector.tensor_copy(stt[:, j:j + 1], ap[:, 0:1])
                nc.sync.dma_start(dbg_st[:], stt[:])

            # ---- phase 4: out = lrelu(bn2(opre) + x); x = xr[0:32] -----
            for c in range(10):
                sl = slice(c * 512, (c + 1) * 512)
                aff = finp.tile([32, 512], f32, tag="fin")
                nc.scalar.activation(out=aff[:], in_=opre[:, sl],
                                     func=AF.Identity, scale=s2[:], bias=b2[:])
                res = finp.tile([32, 512], f32, tag="fin")
                nc.vector.scalar_tensor_tensor(
                    out=res[:], in0=aff[:], scalar=0.0, in1=xr[0:32, sl],
                    op0=ALU.add, op1=ALU.add)
                fo = finp.tile([32, 512], f32, tag="fin")
                nc.vector.scalar_tensor_tensor(
                    out=fo[:], in0=res[:], scalar=SLOPE, in1=res[:],
                    op0=ALU.mult, op1=ALU.max)
                nc.sync.dma_start(out_d[:, sl], fo[:])

    nc.compile()
    return nc


def _get_program():
    if "nc" not in _CACHE:
        _CACHE["nc"] = _build_program()
    return _CACHE["nc"]


def kernel(x, y, W0, g0, b0, Wm, gm, bm, W1, g_out, b_out):
    from concourse.bass_utils import run_bass_kernel_spmd

    x = np.asarray(x, np.float32)
    y = np.asarray(y, np.float32)
    W0 = np.asarray(W0, np.float32)
    Wm = np.asarray(Wm, np.float32)
    W1 = np.asarray(W1, np.float32)

    # host-side weight prep
    V = W1.reshape(Cout, Cin, H, Cout).sum(axis=2)        # (o, i, f)
    V3T = np.ascontiguousarray(V.reshape(Cout * Cin, Cout).T)  # (f=32, oi=1024)
    W0T = np.ascontiguousarray(W0.T)                      # (64, 32)
    WmT = np.ascontiguousarray(Wm.T)                      # (32, 32)
    S = np.zeros((128, 256), np.float32)
    for m in range(8):
        for do in range(4):
            for i in range(32):
                S[do * 32 + i, 32 * m + 4 * m + do] = 1.0
    bnp = np.stack([np.asarray(a, np.float32) for a in
                    (g0, b0, gm, bm, g_out, b_out)], axis=1)  # (32, 6)

    in_maps = []
    for c in range(NCORES):
        nsl = slice(c * NS, (c + 1) * NS)
        # points p = ((b*NS)+nl)*K + k
        xc = np.ascontiguousarray(
            x[:, :, nsl, :].transpose(1, 0, 2, 3).reshape(Cin, P))
        yc = np.ascontiguousarray(
            y[:, :, nsl, :].transpose(1, 0, 2, 3).reshape(Cfeat, P))
        in_maps.append({
            "y0": np.ascontiguousarray(yc[:, :HP]),
            "y1": np.ascontiguousarray(yc[:, HP:]),
            "xrep": np.ascontiguousarray(np.tile(xc, (4, 1))),
            "w0t": W0T, "wmt": WmT, "v3t": V3T, "smask": S, "bnp": bnp,
        })

    nc = _get_program()
    res = run_bass_kernel_spmd(nc, in_maps, list(range(NCORES)))

    out = np.empty((B, Cout, N, K), np.float32)
    for c in range(NCORES):
        oc = res.results[c]["out"]                        # (32, P)
        out[:, :, c * NS:(c + 1) * NS, :] = (
            oc.reshape(Cout, B, NS, K).transpose(1, 0, 2, 3))
    return out


# revision 17
# speedup vs baseline: 1.0306x; 1.0306x over previous
"""Trainium2 Bass kernel for nn_MAK_27401891348771 (gnn_message_passing).

Math (reference):
  t0 = lrelu(BN(W0 @ y));  t1 = lrelu(BN(Wm @ t0));  w = W1 @ t1
  out[b,n,k,o] = sum_{i,h} w[(o,i,h)][b,n,k] * x[b,i,n,k]
  out = lrelu(BN(out) + x)

Key algebraic folds used here:
  - H axis folded into weights on host: V[o,i,f] = sum_h W1[(o,i,h), f]
  - filter apply per point p: out[o,p] = sum_i x[i,p] * A[(o,i),p],
    A = V3 @ t1n  (PE matmul), the x multiply on DVE, the i-reduction as a
    PE matmul against a 0/1 selection mask with PSUM accumulation.
Sharding: N axis across 8 cores (5120 points/core); BN stats via tiny
AllReduce collectives (3x, 256B payloads).
"""

import os
import numpy as np

os.environ.setdefault("MYCRO_LOCAL_CACHE", "1")

B, Cin, Cout, Cfeat, N, K, H = 2, 32, 32, 64, 1024, 20, 4
NCORES = 8
NS = N // NCORES            # 128 n-values per core
P = B * NS * K              # 5120 points per core
PTOT = B * N * K            # 40960 points total
HP = P // 2                 # 2560, y half size
EPS = 1e-5
SLOPE = 0.2

_CACHE = {}
DEBUG_STAGES = False


def _build_program():
    import concourse.bass as bass
    import concourse.tile as tile
    import concourse.bacc as bacc
    from concourse import mybir

    f32 = mybir.dt.float32
    AF = mybir.ActivationFunctionType
    ALU = mybir.AluOpType

    nc = bacc.Bacc(
        "TRN2",
        target_bir_lowering=False,
        debug=False,
        enable_asserts=True,
        num_devices=NCORES,
    )

    # ---- DRAM I/O -------------------------------------------------------
    y0_d = nc.dram_tensor("y0", [64, HP], f32, kind="ExternalInput")
    y1_d = nc.dram_tensor("y1", [64, HP], f32, kind="ExternalInput")
    xr_d = nc.dram_tensor("xrep", [128, P], f32, kind="ExternalInput")
    w0t_d = nc.dram_tensor("w0t", [64, 32], f32, kind="ExternalInput")
    wmt_d = nc.dram_tensor("wmt", [32, 32], f32, kind="ExternalInput")
    v3t_d = nc.dram_tensor("v3t", [32, 1024], f32, kind="ExternalInput")
    sm_d = nc.dram_tensor("smask", [128, 256], f32, kind="ExternalInput")
    bnp_d = nc.dram_tensor("bnp", [32, 6], f32, kind="ExternalInput")
    out_d = nc.dram_tensor("out", [32, P], f32, kind="ExternalOutput")
    if DEBUG_STAGES:
        dbg_t0 = nc.dram_tensor("dbg_t0", [32, P], f32, kind="ExternalOutput")
        dbg_t0n = nc.dram_tensor("dbg_t0n", [32, P], f32, kind="ExternalOutput")
        dbg_t1n = nc.dram_tensor("dbg_t1n", [32, P], f32, kind="ExternalOutput")
        dbg_opre = nc.dram_tensor("dbg_opre", [32, P], f32, kind="ExternalOutput")
        dbg_st = nc.dram_tensor("dbg_st", [32, 12], f32, kind="ExternalOutput")

    RG = [list(range(NCORES))]

    with tile.TileContext(nc, num_cores=NCORES) as tc:
        with (
            tc.tile_pool(name="big", bufs=1) as big,
            tc.tile_pool(name="wts", bufs=1) as wts,
            tc.tile_pool(name="zp", bufs=6) as zp,
            tc.tile_pool(name="fin", bufs=4) as finp,
            tc.tile_pool(name="st", bufs=1) as stp,
            tc.tile_pool(name="psT", bufs=2, space="PSUM") as psT,
            tc.tile_pool(name="psA", bufs=3, space="PSUM") as psA,
            tc.tile_pool(name="psO", bufs=2, space="PSUM") as psO,
            tc.tile_pool(name="dram", bufs=1, space="DRAM") as dram,
        ):
            # ---- persistent SBUF tensors -------------------------------
            y0 = big.tile([64, HP], f32, tag="y0")
            y1 = big.tile([64, HP], f32, tag="y1")
            xr = big.tile([128, P], f32, tag="xr")
            t0 = big.tile([32, P], f32, tag="t0")
            t0n = big.tile([32, P], f32, tag="t0n")
            t1 = big.tile([32, P], f32, tag="t1")
            t1n = big.tile([32, P], f32, tag="t1n")
            opre = big.tile([32, P], f32, tag="opre")
            w0t = wts.tile([64, 32], f32, tag="w0t")
            wmt = wts.tile([32, 32], f32, tag="wmt")
            v3t = wts.tile([32, 1024], f32, tag="v3t")
            smk = wts.tile([128, 256], f32, tag="smk")
            bnp = wts.tile([32, 6], f32, tag="bnp")

            # ---- loads (split for DMA-queue parallelism) ---------------
            for c in range(4):
                nc.sync.dma_start(y0[:, c * 640:(c + 1) * 640],
                                  y0_d[:, c * 640:(c + 1) * 640])
                nc.sync.dma_start(y1[:, c * 640:(c + 1) * 640],
                                  y1_d[:, c * 640:(c + 1) * 640])
            for c in range(8):
                nc.sync.dma_start(xr[:, c * 640:(c + 1) * 640],
                                  xr_d[:, c * 640:(c + 1) * 640])
            nc.sync.dma_start(w0t[:], w0t_d[:])
            nc.sync.dma_start(wmt[:], wmt_d[:])
            nc.sync.dma_start(v3t[:], v3t_d[:])
            nc.sync.dma_start(smk[:], sm_d[:])
            nc.sync.dma_start(bnp[:], bnp_d[:])

            # ---- helpers -----------------------------------------------
            # per-channel sums ride free on the ACT PSUM->SBUF copies via
            # accum_out; stats() only adds the Square pass for sum-of-squares.
            def mkparts(name):
                return stp.tile([32, 16], f32, tag=name, name=name)

            def stats(src, sparts, nsp, nchunks=10):
                """per-channel (sum, sumsq); sparts holds nsp per-chunk sums
                accumulated by earlier ACT copies of src."""
                parts = stp.tile([32, 16], f32, tag=f"sqparts_{src.name}")
                F = P // nchunks
                for c in range(nchunks):
                    scr = finp.tile([32, F], f32, tag="fin")
                    nc.scalar.activation(
                        out=scr[:], in_=src[:, c * F:(c + 1) * F],
                        func=AF.Square, accum_out=parts[:, c:c + 1])
                ssum = stp.tile([32, 1], f32, tag=f"ssum_{src.name}")
                ssq = stp.tile([32, 1], f32, tag=f"ssq_{src.name}")
                nc.vector.tensor_reduce(
                    out=ssum[:], in_=sparts[:, 0:nsp],
                    axis=mybir.AxisListType.X, op=ALU.add)
                nc.vector.tensor_reduce(
                    out=ssq[:], in_=parts[:, 0:nchunks],
                    axis=mybir.AxisListType.X, op=ALU.add)
                return ssum, ssq

            def allreduce_stats(ssum, ssq, idx):
                """AllReduce (32,2) stats; returns SBUF (32,2) of global sums."""
                pack = stp.tile([32, 2], f32, tag=f"arpack{idx}")
                nc.vector.tensor_copy(pack[:, 0:1], ssum[:])
                nc.vector.tensor_copy(pack[:, 1:2], ssq[:])
                bin_ = dram.tile([32, 2], f32, tag=f"arin{idx}")
                bout = dram.tile([32, 2], f32, tag=f"arout{idx}")
                nc.gpsimd.dma_start(bin_[:], pack[:])
                nc.gpsimd.collective_compute(
                    "AllReduce", ALU.add, replica_groups=RG,
                    ins=[bin_.opt()], outs=[bout.opt()])
                glob = stp.tile([32, 2], f32, tag=f"arglob{idx}")
                nc.gpsimd.dma_start(glob[:], bout[:])
                return glob

            def bn_coeffs(glob, gcol, bcol, idx):
                """scale/bias from global (sum,sumsq): s=g*rsqrt(var+eps),
                b = beta - mean*s."""
                mean = stp.tile([32, 1], f32, tag=f"mean{idx}")
                e2 = stp.tile([32, 1], f32, tag=f"e2{idx}")
                nc.scalar.activation(out=mean[:], in_=glob[:, 0:1],
                                     func=AF.Copy, scale=1.0 / PTOT)
                nc.scalar.activation(out=e2[:], in_=glob[:, 1:2],
                                     func=AF.Copy, scale=1.0 / PTOT)
                m2 = stp.tile([32, 1], f32, tag=f"m2{idx}")
                nc.scalar.activation(out=m2[:], in_=mean[:], func=AF.Square)
                varp = stp.tile([32, 1], f32, tag=f"varp{idx}")
                # varp = (e2 - m2) + eps
                nc.vector.scalar_tensor_tensor(
                    out=varp[:], in0=e2[:], scalar=EPS, in1=m2[:],
                    op0=ALU.add, op1=ALU.subtract)
                rv = stp.tile([32, 1], f32, tag=f"rv{idx}")
                nc.vector.reciprocal(rv[:], varp[:])
                isd = stp.tile([32, 1], f32, tag=f"isd{idx}")
                nc.scalar.activation(out=isd[:], in_=rv[:], func=AF.Sqrt)
                s = stp.tile([32, 1], f32, tag=f"s{idx}")
                nc.vector.scalar_tensor_tensor(
                    out=s[:], in0=isd[:], scalar=1.0, in1=bnp[:, gcol:gcol + 1],
                    op0=ALU.mult, op1=ALU.mult)
                ms = stp.tile([32, 1], f32, tag=f"ms{idx}")
                nc.vector.scalar_tensor_tensor(
                    out=ms[:], in0=mean[:], scalar=-1.0, in1=s[:],
                    op0=ALU.mult, op1=ALU.mult)
                bia = stp.tile([32, 1], f32, tag=f"bia{idx}")
                nc.vector.scalar_tensor_tensor(
                    out=bia[:], in0=ms[:], scalar=0.0, in1=bnp[:, bcol:bcol + 1],
                    op0=ALU.add, op1=ALU.add)
                return s, bia

            # ---- phase 1: t0 = W0 @ y ----------------------------------
            t0parts = mkparts("t0parts")
            for h, ysb in ((0, y0), (1, y1)):
                for c in range(5):
                    ps = psT.tile([32, 512], f32, tag="psT")
                    nc.tensor.matmul(ps[:], w0t[:], ysb[:, c * 512:(c + 1) * 512],
                                     start=True, stop=True)
                    nc.scalar.activation(
                        out=t0[:, h * HP + c * 512: h * HP + (c + 1) * 512],
                        in_=ps[:], func=AF.Copy,
                        accum_out=t0parts[:, h * 5 + c: h * 5 + c + 1])

            s0_sum, s0_sq = stats(t0, t0parts, 10)
            g0 = allreduce_stats(s0_sum, s0_sq, 0)
            s0, b0 = bn_coeffs(g0, 0, 1, 0)

            # ---- phase 2: t0n = lrelu(bn0(t0)); t1 = Wm @ t0n ----------
            for c in range(10):
                sl = slice(c * 512, (c + 1) * 512)
                aff = finp.tile([32, 512], f32, tag="fin")
                nc.scalar.activation(out=aff[:], in_=t0[:, sl],
                                     func=AF.Identity, scale=s0[:], bias=b0[:])
                nc.vector.scalar_tensor_tensor(
                    out=t0n[:, sl], in0=aff[:], scalar=SLOPE, in1=aff[:],
                    op0=ALU.mult, op1=ALU.max)
            t1parts = mkparts("t1parts")
            for c in range(10):
                sl = slice(c * 512, (c + 1) * 512)
                ps = psT.tile([32, 512], f32, tag="psT")
                nc.tensor.matmul(ps[:], wmt[:], t0n[:, sl], start=True, stop=True)
                nc.scalar.activation(out=t1[:, sl], in_=ps[:], func=AF.Copy,
                                     accum_out=t1parts[:, c:c + 1])

            s1_sum, s1_sq = stats(t1, t1parts, 10)
            g1 = allreduce_stats(s1_sum, s1_sq, 1)
            s1, b1 = bn_coeffs(g1, 2, 3, 1)

            # ---- phase 3: t1n; filter generate + apply ------------------
            for c in range(10):
                sl = slice(c * 512, (c + 1) * 512)
                aff = finp.tile([32, 512], f32, tag="fin")
                nc.scalar.activation(out=aff[:], in_=t1[:, sl],
                                     func=AF.Identity, scale=s1[:], bias=b1[:])
                nc.vector.scalar_tensor_tensor(
                    out=t1n[:, sl], in0=aff[:], scalar=SLOPE, in1=aff[:],
                    op0=ALU.mult, op1=ALU.max)

            # per group g of 1280 points, col tiles of 512/512/256
            oparts = mkparts("oparts")
            for g in range(4):
                base = g * 1280
                for ci, (c0, F) in enumerate(((0, 512), (512, 512), (1024, 256))):
                    sl = slice(base + c0, base + c0 + F)
                    zs = []
                    for m in range(8):
                        a_ps = psA.tile([128, 512], f32, tag="psA")
                        nc.tensor.matmul(
                            a_ps[:, 0:F], v3t[:, m * 128:(m + 1) * 128],
                            t1n[:, sl], start=True, stop=True)
                        z = zp.tile([128, 512], f32, tag="z")
                        # z = A * xrep
                        nc.vector.scalar_tensor_tensor(
                            out=z[:, 0:F], in0=a_ps[:, 0:F], scalar=1.0,
                            in1=xr[:, sl], op0=ALU.mult, op1=ALU.mult)
                        zs.append(z)
                    o_ps = psO.tile([32, 512], f32, tag="psO")
                    for m in range(8):
                        nc.tensor.matmul(
                            o_ps[:, 0:F], smk[:, m * 32:(m + 1) * 32],
                            zs[m][:, 0:F], start=(m == 0), stop=(m == 7))
                    nc.scalar.activation(out=opre[:, sl], in_=o_ps[:, 0:F],
                                         func=AF.Copy,
                                         accum_out=oparts[:, g * 3 + ci:
                                                          g * 3 + ci + 1])

            s2_sum, s2_sq = stats(opre, oparts, 12)
            g2 = allreduce_stats(s2_sum, s2_sq, 2)
            s2, b2 = bn_coeffs(g2, 4, 5, 2)

            if DEBUG_STAGES:
                for c in range(4):
                    sl = slice(c * 1280, (c + 1) * 1280)
                    nc.sync.dma_start(dbg_t0[:, sl], t0[:, sl])
                    nc.sync.dma_start(dbg_t0n[:, sl], t0n[:, sl])
                    nc.sync.dma_start(dbg_t1n[:, sl], t1n[:, sl])
                    nc.sync.dma_start(dbg_opre[:, sl], opre[:, sl])
                stt = stp.tile([32, 12], f32, tag="dbgst")
                for j, ap in enumerate((g0, s0, b0, g1, s1, b1, g2, s2, b2)):
                    w = ap.shape[1] if len(ap.shape) > 1 else 1
                    nc.vector.tensor_copy(stt[:, j:j + 1], ap[:, 0:1])
                nc.sync.dma_start(dbg_st[:], stt[:])

            # ---- phase 4: out = lrelu(bn2(opre) + x); x = xr[0:32] -----
            for c in range(10):
                sl = slice(c * 512, (c + 1) * 512)
                aff = finp.tile([32, 512], f32, tag="fin")
                nc.scalar.activation(out=aff[:], in_=opre[:, sl],
                                     func=AF.Identity, scale=s2[:], bias=b2[:])
                res = finp.tile([32, 512], f32, tag="fin")
                nc.vector.scalar_tensor_tensor(
                    out=res[:], in0=aff[:], scalar=0.0, in1=xr[0:32, sl],
                    op0=ALU.add, op1=ALU.add)
                fo = finp.tile([32, 512], f32, tag="fin")
                nc.vector.scalar_tensor_tensor(
                    out=fo[:], in0=res[:], scalar=SLOPE, in1=res[:],
                    op0=ALU.mult, op1=ALU.max)
                nc.sync.dma_start(out_d[:, sl], fo[:])

    nc.compile()
    return nc


def _get_program():
    if "nc" not in _CACHE:
        _CACHE["nc"] = _build_program()
    return _CACHE["nc"]


def kernel(x, y, W0, g0, b0, Wm, gm, bm, W1, g_out, b_out):
    from concourse.bass_utils import run_bass_kernel_spmd

    x = np.asarray(x, np.float32)
    y = np.asarray(y, np.float32)
    W0 = np.asarray(W0, np.float32)
    Wm = np.asarray(Wm, np.float32)
    W1 = np.asarray(W1, np.float32)

    # host-side weight prep
    V = W1.reshape(Cout, Cin, H, Cout).sum(axis=2)        # (o, i, f)
    V3T = np.ascontiguousarray(V.reshape(Cout * Cin, Cout).T)  # (f=32, oi=1024)
    W0T = np.ascontiguousarray(W0.T)                      # (64, 32)
    WmT = np.ascontiguousarray(Wm.T)                      # (32, 32)
    S = np.zeros((128, 256), np.float32)
    for m in range(8):
        for do in range(4):
            for i in range(32):
                S[do * 32 + i, 32 * m + 4 * m + do] = 1.0
    bnp = np.stack([np.asarray(a, np.float32) for a in
                    (g0, b0, gm, bm, g_out, b_out)], axis=1)  # (32, 6)

    in_maps = []
    for c in range(NCORES):
        nsl = slice(c * NS, (c + 1) * NS)
        # points p = ((b*NS)+nl)*K + k
        xc = np.ascontiguousarray(
            x[:, :, nsl, :].transpose(1, 0, 2, 3).reshape(Cin, P))
        yc = np.ascontiguousarray(
            y[:, :, nsl, :].transpose(1, 0, 2, 3).reshape(Cfeat, P))
        in_maps.append({
            "y0": np.ascontiguousarray(yc[:, :HP]),
            "y1": np.ascontiguousarray(yc[:, HP:]),
            "xrep": np.ascontiguousarray(np.tile(xc, (4, 1))),
            "w0t": W0T, "wmt": WmT, "v3t": V3T, "smask": S, "bnp": bnp,
        })

    nc = _get_program()
    res = run_bass_kernel_spmd(nc, in_maps, list(range(NCORES)))

    out = np.empty((B, Cout, N, K), np.float32)
    for c in range(NCORES):
        oc = res.results[c]["out"]                        # (32, P)
        out[:, :, c * NS:(c + 1) * NS, :] = (
            oc.reshape(Cout, B, NS, K).transpose(1, 0, 2, 3))
    return out
